# revision 45
# baseline (speedup 1.0000x reference)
"""CharRNN (2-layer LSTM + softmax cross-entropy) Trainium2 kernel.

Sharding: data-parallel over batch — 8 NeuronCores x 8 sequences each;
LSTM/softmax weights replicated; per-core partial loss sums reduced on host.

On-chip design (per core):
  - Feature-major ("transposed") layout everywhere: SBUF partitions carry
    the feature/gate axis, tokens on the free axis.
  - LSTM gates split: big time-batched input projection  pre = W_x^T @ X^T
    per layer, plus per-step weight-stationary matmuls for h_{t-1} @ W_h
    accumulated in PSUM (64 LDW+MM pairs of N=8 per step-layer).
  - All matmuls bf16 (weights converted on-chip), fp32 cell state / sums.
  - CE: logits chunkwise [128 tok x 500 vocab]; exp on ScalarE with
    accumulated row-sums; exp dumped to DRAM bf16; exp(logit_target)
    gathered back by indirect DMA. loss = ln(s/V) + ln(V) - ln(e_tgt).
"""

import sys

sys.path.insert(0, "/opt/trn_rl_repo")

import numpy as np

import concourse.bass as bass
import concourse.mybir as mybir
import concourse.tile as tile
from concourse import bacc
from concourse.bass_utils import run_bass_kernel_spmd
from concourse.masks import make_identity

AF = mybir.ActivationFunctionType
F32 = mybir.dt.float32
BF16 = mybir.dt.bfloat16
I16 = mybir.dt.int16
I32 = mybir.dt.int32

# Recurrent weights in fp8 (e3m4) halve the per-step LDWEIGHTS traffic,
# which is the recurrence bottleneck. Scaling keeps values in fp8 range:
# W_h stored *32, h stored *8; PSUM result is 256x, the activations use
# scale=1/256, and the time-batched "pre" terms are also stored *256.
REC_FP8 = False
# mixed-dtype recurrent matmul: fp8 stationary weights, bf16 moving h.
# If hardware rejects mixed operand dtypes, set False to store an extra
# fp8 copy of h (scaled by H_SCALE).
REC_MIXED = True
RDT = mybir.dt.float8e3 if REC_FP8 else BF16
W_SCALE = 32.0 if REC_FP8 else 1.0
H_SCALE = 1.0 if (REC_MIXED or not REC_FP8) else 8.0
GSCALE = W_SCALE * H_SCALE

# Gate-tile permutation: PSUM/pre position-blocks ordered [i, f, o, j] x4
# so the three sigmoids are one contiguous [128, 96] activation.
# position p (0..15) -> original gate m-tile index (i:0-3 j:4-7 f:8-11 o:12-15)
POS2M = [0, 1, 2, 3, 8, 9, 10, 11, 12, 13, 14, 15, 4, 5, 6, 7]

V = 8000
H = 512
G = 2048  # 4*H
KT = H // 128  # 4 k-tiles over one feature dim
MT = G // 128  # 16 m-tiles over the gate dim
N_CORES = 8
B = 64
BL = B // N_CORES  # sequences per core
VCH = 500  # logits chunk width (<=512: one PSUM bank)
NVC = V // VCH


def _mid(ap3, i, a, b):
    """[128, M, N] AP -> [128, b-a] slice at middle index i."""
    return ap3[:, i:i + 1, a:b].rearrange("p o f -> p (o f)")


def build_program(T=128, ablate=""):
    NT = BL * T  # tokens per core, t-major: token j = t*BL + b
    assert NT % 128 == 0
    NMT = NT // 128
    do_rec = "rec" not in ablate
    do_ce = "ce" not in ablate

    nc = bacc.Bacc("TRN2", target_bir_lowering=False, debug=False,
                   enable_asserts=False, num_devices=N_CORES)

    tok_idx = nc.dram_tensor("tok_idx", [16, NT // 16], I16, kind="ExternalInput")
    tgt_idx = nc.dram_tensor("tgt_idx", [128, NMT], I32, kind="ExternalInput")
    emb = nc.dram_tensor("emb", [V, H], F32, kind="ExternalInput")
    w1_d = nc.dram_tensor("w1", [2 * H, G], F32, kind="ExternalInput")
    b1_d = nc.dram_tensor("b1", [G], F32, kind="ExternalInput")
    w2_d = nc.dram_tensor("w2", [2 * H, G], F32, kind="ExternalInput")
    b2_d = nc.dram_tensor("b2", [G], F32, kind="ExternalInput")
    sw_d = nc.dram_tensor("sw", [H, V], F32, kind="ExternalInput")
    sb_d = nc.dram_tensor("sb", [V], F32, kind="ExternalInput")
    loss_out = nc.dram_tensor("loss_out", [1, 1], F32, kind="ExternalOutput")
    edump = nc.dram_tensor("edump", [NT * V, 1], BF16, kind="Internal")
    edump2d = edump.ap().rearrange("(a b) o -> a (b o)", b=V)

    cast_n = [0]

    with tile.TileContext(nc) as tc:
        with (
            tc.tile_pool(name="big", bufs=4) as big_p,     # x32 / sw_bf
            tc.tile_pool(name="wp", bufs=1) as wp,         # lstm weights
            tc.tile_pool(name="persist", bufs=1) as pp,    # xt / pre / h
            tc.tile_pool(name="stage", bufs=2) as stage_p,
            tc.tile_pool(name="psb", bufs=4, space="PSUM") as psb,
            tc.tile_pool(name="pss", bufs=2, space="PSUM") as pss,
            tc.tile_pool(name="gates", bufs=2) as gates_p,
            tc.tile_pool(name="cell", bufs=2) as cell_p,
            tc.tile_pool(name="lg", bufs=2) as lg_p,
            tc.tile_pool(name="misc", bufs=1) as misc_p,
        ):
            def copy_cast(dst, src):
                # alternate DVE / ACT for fp32->bf16 conversions
                cast_n[0] += 1
                if cast_n[0] % 2:
                    nc.vector.tensor_copy(dst, src)
                else:
                    nc.scalar.copy(dst, src)

            # ----- prologue -----
            ident = misc_p.tile([128, 128], F32, tag="ident")
            make_identity(nc, ident[:])

            idx_t = misc_p.tile([128, NT // 16], I16, tag="idx")
            nc.vector.memset(idx_t[:], 0)
            nc.sync.dma_start(idx_t[:16, :], tok_idx.ap())

            tgt_t = misc_p.tile([128, NMT], I32, tag="tgt")
            nc.sync.dma_start(tgt_t[:], tgt_idx.ap())

            b1_sb = misc_p.tile([128, MT], F32, tag="b1")
            nc.sync.dma_start(b1_sb[:], b1_d.ap().rearrange("(m p) -> p m", p=128))
            nc.vector.tensor_scalar_mul(b1_sb[:], b1_sb[:], GSCALE)
            b2_sb = misc_p.tile([128, MT], F32, tag="b2")
            nc.sync.dma_start(b2_sb[:], b2_d.ap().rearrange("(m p) -> p m", p=128))
            nc.vector.tensor_scalar_mul(b2_sb[:], b2_sb[:], GSCALE)

            # ----- embedding gather + X -> X^T (bf16) via PE transpose -----
            # gathered in halves through a small recycled buffer
            xt = [pp.tile([128, NT], BF16, tag=f"xt{q}", name=f"xt{q}")
                  for q in range(KT)]
            GH = max(NMT // 2, 1)  # token tiles per gather
            for h in range(NMT // GH):
                x32 = pp.tile([128, GH, H], F32, tag="x32", bufs=1,
                              name=f"x32_{h}")
                nc.gpsimd.dma_gather(
                    out_ap=x32[:],
                    in_ap=emb.ap(),
                    idxs_ap=idx_t[:, h * (GH * 8):(h + 1) * (GH * 8)],
                    num_idxs=GH * 128,
                    num_idxs_reg=GH * 128,
                    elem_size=H,
                )
                for g in range(GH):
                    for q in range(KT):
                        ps_t = pss.tile([128, 128], F32, tag="ps_t")
                        nc.tensor.transpose(
                            ps_t[:], _mid(x32[:], g, q * 128, (q + 1) * 128),
                            ident[:])
                        copy_cast(
                            xt[q][:, (h * GH + g) * 128:(h * GH + g + 1) * 128],
                            ps_t[:])

            # ----- LSTM weights: input half -> bf16, recurrent half -> RDT --
            def scaled_cast(dst, src, scale):
                cast_n[0] += 1
                if scale == 1.0:
                    copy_cast(dst, src)
                elif cast_n[0] % 2:
                    nc.vector.tensor_scalar_mul(dst, src, scale)
                else:
                    nc.scalar.mul(dst, src, scale)

            def load_w_bf(w_dram, lname, rec_bufs, fillers=None):
                tiles = []
                for k in range(2 * KT):
                    rec = k >= KT
                    wt = wp.tile([128, G], RDT if rec else BF16, tag=f"w_{k}",
                                 bufs=(1 if k < KT else rec_bufs),
                                 name=f"{lname}_{k}")

                    def unit(k=k, wt=wt, rec=rec):
                        for hh in range(4):
                            st = stage_p.tile([128, G // 4], F32, tag="stage",
                                              name=f"{lname}st{k}_{hh}",
                                              uniquify=True)
                            nc.sync.dma_start(
                                st[:],
                                w_dram.ap()[k * 128:(k + 1) * 128,
                                            hh * (G // 4):(hh + 1) * (G // 4)])
                            scaled_cast(wt[:, hh * (G // 4):(hh + 1) * (G // 4)],
                                        st[:], W_SCALE if rec else 1.0)

                    if fillers is None:
                        unit()
                    else:
                        fillers.append(unit)
                    tiles.append(wt)
                return tiles

            w1_bf = load_w_bf(w1_d, "w1", rec_bufs=2)

            # ----- time-batched input projection -----
            def pre_phase(w_bf, b_sb, rhs_tiles, rhs_is_h, pname):
                # pre[p, m, j] = sum_k W[k, m*128+p] * X^T[k, j] + b[m*128+p]
                pre = pp.tile([128, MT, NT], BF16, tag="pre", name=pname)
                ncw = min(NT, 512)
                nch = NT // ncw
                off = BL if rhs_is_h else 0
                for p in range(MT):
                    m = POS2M[p]
                    for n in range(nch):
                        ps = psb.tile([128, ncw], F32, tag="psb")
                        for k in range(KT):
                            if rhs_is_h:
                                rhs = _mid(rhs_tiles[:], k, off + n * ncw,
                                           off + (n + 1) * ncw)
                            else:
                                rhs = rhs_tiles[k][:, n * ncw:(n + 1) * ncw]
                            nc.tensor.matmul(
                                ps[:], w_bf[k][:, m * 128:(m + 1) * 128], rhs,
                                start=(k == 0), stop=(k == KT - 1))
                        nc.scalar.activation(
                            _mid(pre[:], p, n * ncw, (n + 1) * ncw), ps[:],
                            AF.Identity, bias=b_sb[:, m:m + 1], scale=GSCALE)
                return pre

            pre1 = pre_phase(w1_bf, b1_sb, xt, False, "pre1")

            # ----- recurrence stepper: emits one LSTM time step -----
            def make_stepper(w_bf, pre_fn, hname):
                # ht[p, q, (t+1)*BL + b] holds h_t; cols 0:BL are h_0 = 0
                ht = pp.tile([128, KT, NT + BL], BF16, tag=hname, name=hname)
                nc.vector.memset(ht[:, :, 0:BL], 0)
                c_st = misc_p.tile([128, 4 * BL], F32, tag=f"c_{hname}")
                nc.vector.memset(c_st[:], 0)
                s = 4 * BL
                inv = float(1.0 / GSCALE)

                def step(t):
                    ps_g = pss.tile([128, MT * BL], F32, tag="ps_g")
                    for p in range(MT):
                        for k in range(KT):
                            nc.tensor.matmul(
                                ps_g[:, p * BL:(p + 1) * BL],
                                w_bf[KT + k][:, POS2M[p] * 128:
                                             (POS2M[p] + 1) * 128],
                                _mid(ht[:], k, t * BL, (t + 1) * BL),
                                start=(k == 0), stop=(k == KT - 1))
                    gsb = gates_p.tile([128, MT * BL], F32, tag="gsb")
                    nc.vector.tensor_add(
                        gsb[:].rearrange("p (m b) -> p m b", b=BL),
                        ps_g[:].rearrange("p (m b) -> p m b", b=BL),
                        pre_fn(t))
                    # position layout: [i | f | o | j] so one sigmoid covers
                    # i,f,o and one tanh covers j
                    act = gates_p.tile([128, MT * BL], F32, tag="act")
                    nc.scalar.activation(act[:, 0:3 * s], gsb[:, 0:3 * s],
                                         AF.Sigmoid, scale=inv)
                    nc.scalar.activation(act[:, 3 * s:4 * s],
                                         gsb[:, 3 * s:4 * s], AF.Tanh,
                                         scale=inv)
                    t1 = cell_p.tile([128, s], F32, tag="t1")
                    nc.vector.tensor_mul(t1[:], c_st[:], act[:, s:2 * s])
                    t2 = cell_p.tile([128, s], F32, tag="t2")
                    nc.vector.tensor_mul(t2[:], act[:, 0:s],
                                         act[:, 3 * s:4 * s])
                    nc.vector.tensor_add(c_st[:], t1[:], t2[:])
                    thc = cell_p.tile([128, s], F32, tag="thc")
                    nc.scalar.activation(thc[:], c_st[:], AF.Tanh)
                    nc.vector.tensor_mul(
                        ht[:, :, (t + 1) * BL:(t + 2) * BL],
                        thc[:].rearrange("p (q b) -> p q b", b=BL),
                        act[:, 2 * s:3 * s].rearrange("p (q b) -> p q b", b=BL))

                return ht, step

            # conversion work for layer-2 / softmax weights is emitted as
            # "filler" units inside the rec-1 loop so it lands in the idle
            # slack of the DMA/DVE/ACT streams while PE grinds the recurrence
            # Filler order must respect first-consumption order (emission
            # order IS program order for the dependency tracker): w2 before
            # the first pre2 chunk (pop 16), sb before the first CE chunk
            # (pop ~33), sw column-major so col block n//2 lands before CE
            # chunk n.
            fillers = []
            w2_bf = load_w_bf(w2_d, "w2", rec_bufs=2, fillers=fillers)

            # softmax bias as a single-partition row; added to logits via a
            # K=1 ones-matmul into PSUM so the CE chunks need no DVE work
            sb_bf = misc_p.tile([1, V], BF16, tag="sb_bf")
            ones_l = misc_p.tile([1, 128], BF16, tag="ones_l")
            nc.vector.memset(ones_l[:], 1.0)
            for c in range(8):
                def unit(c=c):
                    w = V // 8
                    for hh in range(2):
                        st = stage_p.tile([128, w // 2], F32, tag="stage",
                                          name=f"sbst{c}_{hh}")
                        lo = c * w + hh * (w // 2)
                        nc.sync.dma_start(
                            st[:1, :],
                            sb_d.ap()[lo:lo + w // 2]
                            .rearrange("(o v) -> o v", o=1))
                        copy_cast(sb_bf[:, lo:lo + w // 2], st[:1, :])
                fillers.append(unit)

            sw_bf = [big_p.tile([128, V], BF16, tag="big", name=f"sw_{q}")
                     for q in range(KT)]
            for c in range(8):
                for q in range(KT):
                    def unit(q=q, c=c):
                        w = V // 8  # 1000
                        for hh in range(2):
                            st = stage_p.tile([128, w // 2], F32, tag="stage",
                                              name=f"swst{q}_{c}_{hh}")
                            lo = c * w + hh * (w // 2)
                            nc.sync.dma_start(
                                st[:],
                                sw_d.ap()[q * 128:(q + 1) * 128,
                                          lo:lo + w // 2])
                            copy_cast(sw_bf[q][:, lo:lo + w // 2], st[:])
                    fillers.append(unit)

            # ----- fused two-layer recurrence pipeline -----
            # layer 2 runs one CH-step chunk behind layer 1; its input
            # projection is computed just-in-time into a 2-slot ring. The
            # interleaved emission lets the two layers' serial cell chains
            # overlap each other's matmul streams.
            CH = T // NMT if NMT else T  # 16 steps = one 128-token tile
            NCH = T // CH

            def pre1_fn(t):
                return pre1[:, :, t * BL:(t + 1) * BL]

            pre2_ring = {}

            def pre2_fn(t):
                c = t // CH
                j = t % CH
                return pre2_ring[c][:, :, j * BL:(j + 1) * BL]

            h1, step1 = make_stepper(w1_bf, pre1_fn, "h1")
            h2, step2 = make_stepper(w2_bf, pre2_fn, "h2")

            def emit_pre2_chunk(c):
                # reuses the xt slots (dead after pre1) as the 2-slot ring
                prc = pp.tile([128, MT, CH * BL], BF16, tag=f"xt{c % 2}",
                              bufs=1, name=f"pre2c{c}")
                pre2_ring[c] = prc
                for p in range(MT):
                    m = POS2M[p]
                    ps = psb.tile([128, CH * BL], F32, tag="psb")
                    for k in range(KT):
                        nc.tensor.matmul(
                            ps[:],
                            w2_bf[k][:, m * 128:(m + 1) * 128],
                            _mid(h1[:], k, BL + c * CH * BL,
                                 BL + (c + 1) * CH * BL),
                            start=(k == 0), stop=(k == KT - 1))
                    nc.scalar.activation(
                        _mid(prc[:], p, 0, CH * BL), ps[:],
                        AF.Identity, bias=b2_sb[:, m:m + 1], scale=GSCALE)

            # ----- logits + CE chunks, popped on rec-2 step boundaries -----
            s_all = misc_p.tile([128, NMT], F32, tag="s_all")
            te = misc_p.tile([128, NMT], BF16, tag="te")
            sums_tiles = {}

            def logits_chunk(m, n):
                if n == 0:
                    sums_tiles[m] = gates_p.tile([128, NVC], F32, tag="sums",
                                                 name=f"sums{m}")
                sums = sums_tiles[m]
                ps_l = psb.tile([128, VCH], F32, tag="psb")
                nc.tensor.matmul(ps_l[:], ones_l[:],
                                 sb_bf[:, n * VCH:(n + 1) * VCH],
                                 start=True, stop=False)
                for k in range(KT):
                    nc.tensor.matmul(
                        ps_l[:],
                        _mid(h2[:], k, BL + m * 128, BL + (m + 1) * 128),
                        sw_bf[k][:, n * VCH:(n + 1) * VCH],
                        start=False, stop=(k == KT - 1))
                exc = lg_p.tile([128, VCH], BF16, tag="exc")
                nc.scalar.activation(exc[:], ps_l[:], AF.Exp,
                                     accum_out=sums[:, n:n + 1])
                nc.sync.dma_start(
                    edump2d[m * 128:(m + 1) * 128, n * VCH:(n + 1) * VCH],
                    exc[:])
                if n == NVC - 1:
                    nc.vector.reduce_sum(s_all[:, m:m + 1], sums[:],
                                         axis=mybir.AxisListType.X)

            mid = {}
            if do_ce:
                for m in range(NMT):
                    for n in range(NVC):
                        key = (m + 1) * CH + 1 + n
                        mid[key] = (lambda m=m, n=n: logits_chunk(m, n))

            def after_rec2_step(s2):
                if (s2 + 1) in mid:
                    mid.pop(s2 + 1)()

            if do_rec:
                def pop_fillers(k=2):
                    for _ in range(k):
                        if fillers:
                            fillers.pop(0)()

                for j in range(CH):
                    step1(j)
                    pop_fillers()
                emit_pre2_chunk(0)
                for c in range(1, NCH):
                    for j in range(CH):
                        step1(c * CH + j)
                        step2((c - 1) * CH + j)
                        pop_fillers()
                        after_rec2_step((c - 1) * CH + j)
                    emit_pre2_chunk(c)
                for j in range(CH):
                    step2((NCH - 1) * CH + j)
                    pop_fillers()
                    after_rec2_step((NCH - 1) * CH + j)
            else:
                nc.vector.memset(h1[:], 0)
                nc.vector.memset(h2[:], 0)
            for f in fillers:
                f()
            for key in sorted(mid):
                mid.pop(key)()

            if not do_ce:
                nc.vector.memset(s_all[:], 1.0)
            lse = misc_p.tile([128, NMT], F32, tag="lse")
            nc.scalar.activation(lse[:], s_all[:], AF.Ln, scale=float(1.0 / V))

            for c in range(NMT):
                nc.gpsimd.indirect_dma_start(
                    out=te[:, c:c + 1],
                    out_offset=None,
                    in_=edump.ap(),
                    in_offset=bass.IndirectOffsetOnAxis(ap=tgt_t[:, c:c + 1],
                                                        axis=0))
            if not do_ce:
                nc.vector.memset(te[:], 1.0)
            lt = misc_p.tile([128, NMT], F32, tag="lt")
            nc.scalar.activation(lt[:], te[:], AF.Ln)

            loss = misc_p.tile([128, NMT], F32, tag="loss")
            nc.vector.tensor_sub(loss[:], lse[:], lt[:])
            nc.vector.tensor_scalar_add(loss[:], loss[:], float(np.log(V)))
            lcol = misc_p.tile([128, 1], F32, tag="lcol")
            nc.vector.reduce_sum(lcol[:], loss[:], axis=mybir.AxisListType.X)
            onec = misc_p.tile([128, 1], F32, tag="onec")
            nc.vector.memset(onec[:], 1.0)
            ps_f = pss.tile([1, 1], F32, tag="ps_t")
            nc.tensor.matmul(ps_f[:], lcol[:], onec[:], start=True, stop=True)
            lone = misc_p.tile([1, 1], F32, tag="lone")
            nc.vector.tensor_copy(lone[:], ps_f[:])
            nc.sync.dma_start(loss_out.ap(), lone[:])

    nc.compile()
    return nc


def make_in_maps(input_data, targets, embedding, W1, b1, W2, b2,
                 softmax_w, softmax_b, T=128):
    NT = BL * T
    NMT = NT // 128
    in_maps = []
    ar = np.arange(NT)
    for c in range(N_CORES):
        toks = np.asarray(input_data[c * BL:(c + 1) * BL, :T]).astype(np.int64)
        tgts = np.asarray(targets[c * BL:(c + 1) * BL, :T]).astype(np.int64)
        tok_flat = toks.T.reshape(-1)  # t-major: j = t*BL + b
        tgt_flat = tgts.T.reshape(-1)
        tok_i16 = np.zeros((16, NT // 16), np.int16)
        tok_i16[ar % 16, ar // 16] = tok_flat.astype(np.int16)
        tgt_i32 = np.zeros((128, NMT), np.int32)
        tgt_i32[ar % 128, ar // 128] = (ar.astype(np.int64) * V
                                        + tgt_flat).astype(np.int32)
        in_maps.append({
            "tok_idx": tok_i16,
            "tgt_idx": tgt_i32,
            "emb": np.ascontiguousarray(embedding, dtype=np.float32),
            "w1": np.ascontiguousarray(W1, dtype=np.float32),
            "b1": np.ascontiguousarray(b1, dtype=np.float32),
            "w2": np.ascontiguousarray(W2, dtype=np.float32),
            "b2": np.ascontiguousarray(b2, dtype=np.float32),
            "sw": np.ascontiguousarray(softmax_w, dtype=np.float32),
            "sb": np.ascontiguousarray(softmax_b, dtype=np.float32),
        })
    return in_maps


_PROGRAM_CACHE = {}


def _get_program(T=128):
    if T not in _PROGRAM_CACHE:
        _PROGRAM_CACHE[T] = build_program(T)
    return _PROGRAM_CACHE[T]


def kernel(input_data, targets, embedding, W1, b1, W2, b2, softmax_w,
           softmax_b):
    T = 128
    nc = _get_program(T)
    in_maps = make_in_maps(input_data, targets, embedding, W1, b1, W2, b2,
                           softmax_w, softmax_b, T=T)
    res = run_bass_kernel_spmd(nc, in_maps, list(range(N_CORES)), trace=False)
    total = np.float64(0.0)
    for r in res.results:
        total += np.float64(r["loss_out"][0, 0])
    return np.float32(total / (B * 128))
